# revision 32
# baseline (speedup 1.0000x reference)
"""Masked dot-product attention on 8 Trainium2 NeuronCores (Bass/Tile).

Problem: queries/keys/values [32, 1024, 128] f32, valid_lens [32] i32.
  out = softmax(mask(Q K^T / sqrt(128))) V        (key-padding prefix mask)

v2 strategy — piece-parallel, ACT-bound pipeline:
  * Attention numerator (sum_k p_k v_k) and denominator (sum_k p_k) are
    ADDITIVE over key chunks, so a batch's key range can be cut into
    pieces processed on different slots/cores; the host sums partial
    numerators/denominators, divides, and transposes.
  * The planner cuts the 32 batches' chunk-needs (sum = ceil-per-128 of
    valid_lens) into pieces that EXACTLY fill an SPMD-identical slot
    profile across the 8 cores — per-core work drops to
    ceil(total_chunks/8) with zero padding when an exact packing exists
    (for the fixed harness input: 17 chunks/core vs 20 for slot-max).
  * All matmul operands are bf16 (same PE rate as f32r, half the DMA and
    SBUF), PSUM accumulation f32. exp runs on ACT (f32 PSUM in -> bf16
    out) — ACT is the bottleneck engine, so it does NOTHING else.
  * Scores are computed transposed: S^T[k, q] = K_chunk^T-as-lhsT @ Q^T;
    the prefix key mask is per-partition, folded into exp via ACT bias.
  * Per piece: AV accumulates V_chunk-as-lhsT @ p in PSUM across chunks;
    the denominator is a DVE bf16 add-tree over the piece's p tiles plus
    ONE ones-column matmul, keeping PE cost ~1/2 matmul per chunk.
  * Epilogue copies run on GpSimd (idle engine), out is DMA'd as bf16,
    sums as f32. Input DMAs are per-slot packed segments (kt|vp|qt), one
    trigger each, smallest slot first so the PE starts within ~1us.
  * PE program order: scores for chunk i+2 are emitted BEFORE the
    (deferred by one iteration) AV of chunk i-1 and any slot epilogue,
    so the in-order PE queue always produces ACT's next input first.
"""

import math

import numpy as np
import ml_dtypes

import concourse.bacc as bacc
import concourse.bass as bass
import concourse.mybir as mybir
import concourse.tile as tile
from concourse.bass_utils import run_bass_kernel_spmd

B, Q, K, D = 32, 1024, 1024, 128
N_CORES = 8
PART = 128
NCHUNK = K // PART
MASK_BIAS = -1.0e6
INV_SQRT_D = 1.0 / math.sqrt(D)
F32 = mybir.dt.float32
BF16 = mybir.dt.bfloat16
NPBF16 = ml_dtypes.bfloat16

_NC_CACHE: dict = {}
_PLAN_CACHE: dict = {}


# ---------------------------------------------------------------- planner
def _decompose(caps, sizes, counts):
    """Cut caps into parts drawn from `sizes` with exactly counts[s] parts
    of size s overall. Returns list of part-lists per cap, or None."""
    order = sorted(range(len(caps)), key=lambda i: -caps[i])
    sizes = sorted(sizes, reverse=True)
    comp_cache = {}

    def comps(c):
        if c in comp_cache:
            return comp_cache[c]
        out = []

        def rec(c, maxs, cur):
            if c == 0:
                out.append(tuple(cur))
                return
            for s in sizes:
                if s > maxs or s > c:
                    continue
                cur.append(s)
                rec(c - s, s, cur)
                cur.pop()

        rec(c, max(sizes), [])
        comp_cache[c] = out
        return out

    res = [None] * len(caps)
    cnt = dict(counts)
    nodes = [0]

    def dfs(i):
        nodes[0] += 1
        if nodes[0] > 120000:
            return False
        if i == len(order):
            return all(v == 0 for v in cnt.values())
        b = order[i]
        for comp in comps(caps[b]):
            ok = True
            for s in comp:
                cnt[s] -= 1
                if cnt[s] < 0:
                    ok = False
            if ok and dfs(i + 1):
                res[b] = list(comp)
                return True
            for s in comp:
                cnt[s] += 1
        return False

    return res if dfs(0) else None


def _partitions(total, max_part, max_count):
    results = []

    def rec(rem, maxp, cur):
        if rem == 0:
            results.append(tuple(cur))
            return
        if len(cur) >= max_count:
            return
        for p in range(min(maxp, rem), 0, -1):
            cur.append(p)
            rec(rem - p, p, cur)
            cur.pop()

    rec(total, max_part, [])
    results.sort(key=lambda t: (len(t), [-x for x in t]))
    return results


def plan(lens):
    """-> (profile, assign): profile = per-core slot caps in processing
    order (pyramid); assign[core][slot] = (batch, chunk_start, n) or None
    (padded slot, fully masked). Only FULL 128-key chunks go on device;
    each batch's remainder keys (len % 128) are handled on the host."""
    lens = np.asarray(lens).astype(np.int64)
    nb = len(lens)
    caps = [int(c) for c in np.minimum(lens // PART, NCHUNK)]
    nz = [b for b in range(nb) if caps[b] > 0]
    caps_nz = [caps[b] for b in nz]
    total = sum(caps_nz)
    if total == 0:
        return (), [[] for _ in range(N_CORES)]

    found = None
    base_T = -(-total // N_CORES)
    for T in range(base_T, base_T + 2):
        for prof in _partitions(T, min(NCHUNK, T), 8):
            sizes = sorted(set(prof), reverse=True)
            if min(caps_nz) < min(sizes):
                continue
            counts = {s: N_CORES * prof.count(s) for s in sizes}
            dec = _decompose(caps_nz, sizes, counts)
            if dec is not None:
                found = (prof, dec)
                break
        if found:
            break

    if found is not None:
        prof, dec = found
        pieces_by_size = {}
        for j, b in enumerate(nz):
            start = 0
            for part in sorted(dec[j], reverse=True):
                pieces_by_size.setdefault(part, []).append((b, start, part))
                start += part
        # processing order: smallish first (fast pipeline start), big in
        # the middle, absolute smallest last (shortest drain tail)
        asc = sorted(prof)
        rest = asc[1:]
        order_prof = tuple(rest[::2] + rest[1::2][::-1] + asc[:1])
        assign = [[] for _ in range(N_CORES)]
        idx = {s: 0 for s in pieces_by_size}
        for cap in order_prof:
            for core in range(N_CORES):
                lst = pieces_by_size.get(cap)
                if lst is not None and idx.get(cap, 0) < len(lst):
                    assign[core].append(lst[idx[cap]])
                    idx[cap] += 1
                else:
                    assign[core].append(None)
        return order_prof, assign

    # fallback: classic slot-max scheme (always feasible, mask padding)
    bpc = -(-len(nz) // N_CORES)
    order = [int(j) for j in np.argsort([-c for c in caps_nz], kind="stable")]
    groups = [order[s * N_CORES:(s + 1) * N_CORES] for s in range(bpc)]
    prof_caps = [max(caps_nz[j] for j in g) for g in groups]
    slot_order = sorted(range(bpc), key=lambda s: prof_caps[s])
    order_prof = tuple(prof_caps[s] for s in slot_order)
    assign = [[] for _ in range(N_CORES)]
    for s in slot_order:
        for core in range(N_CORES):
            if core < len(groups[s]):
                j = groups[s][core]
                assign[core].append((nz[j], 0, caps_nz[j]))
            else:
                assign[core].append(None)
    return order_prof, assign


# ----------------------------------------------------------- bass program
def build_nc(profile: tuple) -> bass.Bass:
    nc = bacc.Bacc()
    S = len(profile)
    tot = sum(profile)

    # per-slot packed input segment: [ kt (cap*128) | vp (cap*128) | qt (1024) ]
    ins_d = [
        nc.declare_dram_parameter(
            f"ins{s}", [PART, 2 * profile[s] * PART + Q], BF16, isOutput=False
        )
        for s in range(S)
    ]
    mb_d = nc.declare_dram_parameter("mb", [PART, tot], F32, isOutput=False)
    out_d = nc.declare_dram_parameter("out", [S, PART, Q], BF16, isOutput=True)
    # per-piece partition-wise sums of p (host reduces over partitions)
    gsum_d = nc.declare_dram_parameter("gsum", [S, PART, Q], BF16, isOutput=True)

    stream = [(s, c) for s, cap in enumerate(profile) for c in range(cap)]
    N = len(stream)
    pos_of = {}
    off = 0
    for s, cap in enumerate(profile):
        for c in range(cap):
            pos_of[(s, c)] = off + c
        off += cap
    last_of_slot = {s: sum(profile[:s + 1]) - 1 for s in range(S)}

    with tile.TileContext(nc) as tc:
        maxcap = max(profile)
        with (
            tc.tile_pool(name="ins", bufs=3) as insp,
            tc.tile_pool(name="consts", bufs=1) as consts,
            tc.tile_pool(name="probs", bufs=8) as probs,
            tc.tile_pool(name="accs", bufs=6) as accsp,
            tc.tile_pool(name="outsb", bufs=3) as outsbp,
            tc.tile_pool(name="ps_s", bufs=2, space="PSUM") as ps_s,
            tc.tile_pool(name="ps_out", bufs=2, space="PSUM") as ps_out,
        ):
            # Input DMAs: emitted just-in-time AND through a 3-deep
            # rotating uniform tile pool. The WAR dependency of buffer
            # reuse (slot s's DMA waits slot s-3's readers) forces the
            # scheduler to break DMA completion barriers apart, so early
            # consumers aren't gated on later slots' transfers.
            ins_sb = []
            dma_done = [False] * S

            def load_ins(s):
                # alternate DMA rings so each slot's completion barrier
                # covers only its own ring's (fewer, earlier) transfers
                if 0 <= s < S and not dma_done[s]:
                    dma_done[s] = True
                    t = insp.tile(
                        [PART, 2 * maxcap * PART + Q], BF16,
                        tag="ins", name=f"ins{s}",
                    )
                    ins_sb.append(t)
                    w = 2 * profile[s] * PART + Q
                    eng = nc.sync if s % 2 == 0 else nc.gpsimd
                    eng.dma_start(out=t[:, :w], in_=ins_d[s][:, :])

            load_ins(0)
            # mask biases ride the Activation ring: one-off, and the
            # scalar queue is idle until the first exp anyway
            mb_sb = consts.tile([PART, tot], F32, name="mb")
            nc.scalar.dma_start(out=mb_sb, in_=mb_d[:, :])
            load_ins(1)

            def kt_ap(s, c):
                return ins_sb[s][:, c * PART:(c + 1) * PART]

            def vp_ap(s, c):
                cap = profile[s]
                return ins_sb[s][:, (cap + c) * PART:(cap + c + 1) * PART]

            def qt_ap(s, lo, hi):
                cap = profile[s]
                return ins_sb[s][:, 2 * cap * PART + lo:2 * cap * PART + hi]

            def s_mms(i):
                s, c = stream[i]
                sp = ps_s.tile([PART, Q], F32, tag="s", name=f"s{i}")
                kw = kt_ap(s, c)
                for h in range(2):
                    nc.tensor.matmul(
                        sp[:, h * 512:(h + 1) * 512],
                        kw,
                        qt_ap(s, h * 512, (h + 1) * 512),
                        start=True,
                        stop=True,
                    )
                return sp

            s_tiles = {}
            for j in range(min(2, N)):
                s_tiles[j] = s_mms(j)

            p_tiles = {}          # stream index -> p tile (bf16)
            acc_state = {}        # slot -> list of pending partial tiles
            out_ps_of = {}        # slot -> PSUM accumulator
            finish1 = []          # slots: emit AV-done epilogue copy
            finish2 = []          # slots: emit sums matmul + sums copy

            def emit_av(j):
                s, c = stream[j]
                cap = profile[s]
                if c == 0:
                    out_ps_of[s] = ps_out.tile(
                        [PART, Q], F32, tag="out", name=f"out_s{s}"
                    )
                op = out_ps_of[s]
                vw = vp_ap(s, c)
                pj = p_tiles[j]
                for h in range(2):
                    nc.tensor.matmul(
                        op[:, h * 512:(h + 1) * 512],
                        vw,
                        pj[:, h * 512:(h + 1) * 512],
                        start=(c == 0),
                        stop=(c == cap - 1),
                    )

            def emit_adds(j):
                # DVE bf16 add-tree over the slot's p tiles; leaves the
                # final reduced tile in acc_state[s][0] when slot complete.
                s, c = stream[j]
                cap = profile[s]
                st = acc_state.setdefault(s, [])
                st.append((p_tiles[j], 1))
                # merge equal-weight neighbors (binary counter) --> log tree
                while len(st) >= 2 and (
                    st[-1][1] == st[-2][1] or c == cap - 1
                ):
                    (a, na), (b_, nb) = st[-2], st[-1]
                    t = accsp.tile(
                        [PART, Q], BF16, tag="acc", name=f"acc{j}_{len(st)}"
                    )
                    nc.vector.tensor_add(t, a, b_)
                    st[-2:] = [(t, na + nb)]

            def emit_finish1(s):
                # out accumulator -> SBUF bf16 (only ACT/DVE read PSUM).
                # The final slot splits the copy across ACT (idle after the
                # last exp) and DVE to shorten the drain tail; its DMA goes
                # out on the then-idle SP ring instead of GpSimd's.
                osb = outsbp.tile([PART, Q], BF16, tag="osb", name=f"osb{s}")
                if s == S - 1:
                    # drain tail: gsum (ready at last exp) goes out first on
                    # the serial ring, then the out halves split across ACT
                    # (idle after the last exp) and DVE, each DMA'd as soon
                    # as copied
                    emit_finish2(s)
                    nc.scalar.copy(osb[:, 0:512], out_ps_of[s][:, 0:512])
                    nc.sync.dma_start(out=out_d[s][:, 0:512], in_=osb[:, 0:512])
                    nc.vector.tensor_copy(osb[:, 512:], out_ps_of[s][:, 512:])
                    nc.sync.dma_start(out=out_d[s][:, 512:], in_=osb[:, 512:])
                elif s == S - 2:
                    nc.vector.tensor_copy(osb, out_ps_of[s])
                    nc.sync.dma_start(out=out_d[s], in_=osb)
                else:
                    nc.vector.tensor_copy(osb, out_ps_of[s])
                    nc.gpsimd.dma_start(out=out_d[s], in_=osb)

            gsum_done = set()

            def emit_finish2(s):
                # denominator: DMA the piece's partition-wise p-sum tile;
                # the host reduces over the 128 key partitions
                if s in gsum_done:
                    return
                gsum_done.add(s)
                cap = profile[s]
                rhs = acc_state[s][0][0] if cap > 1 else p_tiles[last_of_slot[s]]
                if s >= S - 2:
                    nc.sync.dma_start(out=gsum_d[s], in_=rhs)
                else:
                    nc.gpsimd.dma_start(out=gsum_d[s], in_=rhs)

            for i in range(N + 2):
                if i < N:
                    s, c = stream[i]
                    if c == 0:
                        load_ins(s + 2)
                    p = probs.tile([PART, Q], BF16, tag="p", name=f"p{i}")
                    nc.scalar.activation(
                        p,
                        s_tiles.pop(i),
                        mybir.ActivationFunctionType.Exp,
                        bias=mb_sb[:, pos_of[(s, c)]:pos_of[(s, c)] + 1],
                        scale=INV_SQRT_D,
                    )
                    p_tiles[i] = p
                    if i + 2 < N:
                        s_tiles[i + 2] = s_mms(i + 2)
                # deferred slot finishes (in order: frees PSUM earliest)
                for s_ in finish1:
                    emit_finish1(s_)
                finish1.clear()
                # deferred AV of previous chunk
                if 0 <= i - 1 < N:
                    emit_av(i - 1)
                    if profile[stream[i - 1][0]] > 1:
                        emit_adds(i - 1)
                    sl, cl = stream[i - 1]
                    if cl == profile[sl] - 1:
                        finish1.append(sl)
                for s_ in finish2:
                    emit_finish2(s_)
                finish2.clear()
                if 0 <= i - 1 < N:
                    sl, cl = stream[i - 1]
                    if cl == profile[sl] - 1:
                        finish2.append(sl)
            for s_ in finish1:
                emit_finish1(s_)
            for s_ in finish2:
                emit_finish2(s_)

    nc.compile()
    return nc


# ------------------------------------------------------------------ host
def _host_tail(q, k, v, lens, num, den):
    """Add the remainder keys (beyond the last full 128-chunk) exactly."""
    for b in range(len(lens)):
        s0 = min(int(lens[b]) // PART, NCHUNK) * PART
        L = int(lens[b])
        if s0 >= L:
            continue
        sc = (q[b] @ k[b, s0:L].T).astype(np.float32) * INV_SQRT_D
        p = np.exp(sc)                      # [Q, r]
        num[b] += (p @ v[b, s0:L]).T        # [D, Q]
        den[b] += p.sum(axis=1)[None, :]


def _prep_core_inputs(core, profile, assign, qT, kT, v, lens):
    """Build the per-slot packed input segments + mask for one core."""
    S = len(profile)
    tot = sum(profile)
    ins = []
    mb = np.empty((PART, tot), np.float32)
    pos = 0
    for s in range(S):
        cap = profile[s]
        seg = np.zeros((PART, 2 * cap * PART + Q), NPBF16)
        pc = assign[core][s]
        if pc is not None:
            b, st, n = pc
            k0, k1 = st * PART, (st + n) * PART
            seg[:, 0:n * PART] = kT[b][:, k0:k1]
            seg[:, cap * PART:(cap + n) * PART] = (
                v[b][k0:k1]
                .reshape(n, PART, D)
                .transpose(1, 0, 2)
                .reshape(PART, n * PART)
            )
            seg[:, 2 * cap * PART:] = qT[b]
            # device chunks are always fully valid; mask only padding
            mb[:, pos:pos + n] = 0.0
            mb[:, pos + n:pos + cap] = MASK_BIAS
        else:
            mb[:, pos:pos + cap] = MASK_BIAS
        ins.append(seg)
        pos += cap
    m = {f"ins{s}": ins[s] for s in range(S)}
    m["mb"] = np.ascontiguousarray(mb)
    return m


def kernel(queries, keys, values, valid_lens):
    q = np.asarray(queries, dtype=np.float32)
    k = np.asarray(keys, dtype=np.float32)
    v = np.asarray(values, dtype=np.float32)
    lens = np.asarray(valid_lens).astype(np.int64).reshape(B)

    key = tuple(int(x) for x in lens)
    if key not in _PLAN_CACHE:
        _PLAN_CACHE[key] = plan(lens)
    profile, assign = _PLAN_CACHE[key]

    if len(profile) > 0:
        if profile not in _NC_CACHE:
            _NC_CACHE[profile] = build_nc(profile)
        nc = _NC_CACHE[profile]

    num = np.zeros((B, PART, Q), np.float32)   # [v, q] per batch
    den = np.zeros((B, 1, Q), np.float32)

    if len(profile) > 0:
        qT = np.ascontiguousarray(q.transpose(0, 2, 1)).astype(NPBF16)
        kT = np.ascontiguousarray(k.transpose(0, 2, 1)).astype(NPBF16)
        vb = v.astype(NPBF16)
        in_maps = [
            _prep_core_inputs(core, profile, assign, qT, kT, vb, lens)
            for core in range(N_CORES)
        ]
        res = run_bass_kernel_spmd(nc, in_maps, list(range(N_CORES)))
        for core in range(N_CORES):
            co = res.results[core]["out"]    # [S, 128, 1024] bf16
            cs = res.results[core]["gsum"]   # [S, 128, 1024] bf16
            for s, pc in enumerate(assign[core]):
                if pc is None:
                    continue
                b = pc[0]
                num[b] += co[s].astype(np.float32)
                den[b] += cs[s].astype(np.float32).sum(axis=0, keepdims=True)

    _host_tail(q, k, v, lens, num, den)
    return np.ascontiguousarray((num / den).transpose(0, 2, 1))
